# revision 2
# baseline (speedup 1.0000x reference)
"""Causal multi-head attention block (GPT-style) on 8 TRN2 NeuronCores.

Sharding: core (b, g) = batch b in {0,1} x head-group g in {0..3} (4 heads of
dh=64 each). Megatron-style: each core computes q/k/v projections for its 256
channels, attention for its 4 heads, and a partial c_proj using its 256 rows of
W_proj. Host sums the 4 partial projections per batch (+ bias terms).

On-core dataflow (all matmuls in float32r):
  qT,kT = (W_qk stationary) @ xT        -> [512, 2048]  (q pre-scaled by 1/8)
  v     = (xT stationary) @ W_v         -> [2048, 4*65] (ones column appended)
  sT    = kT_tile.T @ qT_slice          -> scores transposed [j, i], 2 heads
                                           packed in the PE array (K=64 each)
  u     = exp(sT)  (no max-subtraction: scores are O(3); causal tiles only,
                    diagonal tiles masked by elementwise multiply)
  av    = (v|1 stationary) @ u          -> [65, 512]: rows 0-63 unnormalized
                                           attn-out^T, row 64 = softmax sums
  aT    = av[0:64] * (1/av[64]) broadcast via PE ones-column matmul
  out   = (aT stationary) @ W_proj_rows -> partial [2048, 1024]
"""

import sys

try:
    import concourse  # noqa: F401
except ImportError:
    sys.path.insert(0, "/opt/trn_rl_repo")

from contextlib import ExitStack

import numpy as np

import concourse.bass as bass
import concourse.tile as tile
from concourse import bacc, mybir
from concourse.bass_utils import run_bass_kernel_spmd

F32 = mybir.dt.float32
F32R = mybir.dt.float32r
EXP = mybir.ActivationFunctionType.Exp
MUL = mybir.AluOpType.mult

B, T, D = 2, 2048, 1024
HG, DH = 4, 64          # heads per core, head dim
CQK = 512               # q+k channels per core
CV = 256                # v channels per core
KT = D // 128           # contraction tiles of the projections
TS = 512                # t-slice width
NTS = T // TS
NT128 = T // 128


def build():
    nc = bacc.Bacc(None)

    xT_in = nc.dram_tensor("xT", [KT, 128, T], F32R, kind="ExternalInput")
    wqk_in = nc.dram_tensor("wqk", [KT, 128, CQK], F32R, kind="ExternalInput")
    wv_in = nc.dram_tensor("wv", [KT, 128, CV], F32R, kind="ExternalInput")
    wp_in = nc.dram_tensor("wp", [2, 128, D], F32R, kind="ExternalInput")
    bias_in = nc.dram_tensor("bqk", [128, 4], F32, kind="ExternalInput")
    mask_in = nc.dram_tensor("mask", [128, 896], F32R, kind="ExternalInput")
    out_dram = nc.dram_tensor("out", [T, D], F32, kind="ExternalOutput")

    with ExitStack() as ctx:
        tc = ctx.enter_context(tile.TileContext(nc))

        const = ctx.enter_context(tc.tile_pool(name="const", bufs=1))
        big = ctx.enter_context(tc.tile_pool(name="big", bufs=1))
        upool = ctx.enter_context(tc.tile_pool(name="upool", bufs=3))
        atup = ctx.enter_context(tc.tile_pool(name="atup", bufs=3))
        rows = ctx.enter_context(tc.tile_pool(name="rows", bufs=2))
        rbcp = ctx.enter_context(tc.tile_pool(name="rbcp", bufs=2))
        outp = ctx.enter_context(tc.tile_pool(name="outp", bufs=3))

        ps_s = ctx.enter_context(tc.tile_pool(name="ps_s", bufs=2, space="PSUM"))
        ps_av = ctx.enter_context(tc.tile_pool(name="ps_av", bufs=2, space="PSUM"))
        ps_mm = ctx.enter_context(tc.tile_pool(name="ps_mm", bufs=2, space="PSUM"))

        # constants
        bias_sb = const.tile([128, 4], F32, tag="bias")
        nc.sync.dma_start(out=bias_sb[:], in_=bias_in[:])
        mask_sb = const.tile([128, 896], F32R, tag="mask")
        nc.sync.dma_start(out=mask_sb[:], in_=mask_in[:])
        ones128 = const.tile([128, HG], F32, tag="ones128")
        nc.vector.memset(ones128[:], 1.0)
        ones_r = const.tile([1, 64], F32R, tag="ones_r")
        onesf = const.tile([1, 64], F32, tag="onesf")
        nc.vector.memset(onesf[:], 1.0)
        nc.vector.tensor_copy(ones_r[:], onesf[:])

        # persistent intermediates
        qkT = [big.tile([128, T], F32R, tag=f"qkT{c}", name=f"qkT{c}") for c in range(4)]
        V = [big.tile([128, HG * (DH + 1)], F32R, tag=f"V{t}", name=f"V{t}") for t in range(NT128)]
        aT = [big.tile([128, T], F32R, tag=f"aT{c}", name=f"aT{c}") for c in range(2)]
        wp = [big.tile([128, D], F32R, tag=f"wp{c}", name=f"wp{c}") for c in range(2)]
        for c in range(2):
            nc.sync.dma_start(out=wp[c][:], in_=wp_in[c])

        # ---- phase A: projections, streamed over t-slices -----------------
        with (
            tc.tile_pool(name="xrp", bufs=2) as xrp,
            tc.tile_pool(name="wqkp", bufs=1) as wqkp,
            tc.tile_pool(name="wvp", bufs=1) as wvp,
        ):
            wqk = []
            wv = []
            for k in range(KT):
                w1 = wqkp.tile([128, CQK], F32R, tag=f"wqk{k}")
                nc.sync.dma_start(out=w1[:], in_=wqk_in[k])
                wqk.append(w1)
                w2 = wvp.tile([128, CV], F32R, tag=f"wv{k}")
                nc.sync.dma_start(out=w2[:], in_=wv_in[k])
                wv.append(w2)

            for ts in range(NTS):
                xr = []
                for k in range(KT):
                    x1 = xrp.tile([128, TS], F32R, tag=f"xr{k}")
                    nc.sync.dma_start(out=x1[:], in_=xT_in[k][:, ts * TS:(ts + 1) * TS])
                    xr.append(x1)

                # qT / kT (channel-tiles on partitions)
                for ct in range(4):
                    ps = ps_mm.tile([128, TS], F32, tag="mm")
                    for k in range(KT):
                        nc.tensor.matmul(ps[:], wqk[k][:, ct * 128:(ct + 1) * 128],
                                         xr[k][:], start=(k == 0), stop=(k == KT - 1))
                    scale = 0.125 if ct < 2 else 1.0
                    nc.vector.tensor_scalar(
                        qkT[ct][:, ts * TS:(ts + 1) * TS], ps[:],
                        scale, bias_sb[:, ct:ct + 1],
                        op0=mybir.AluOpType.mult, op1=mybir.AluOpType.add)

                # v natural orientation
                for sub in range(4):
                    t128 = ts * 4 + sub
                    ps = ps_mm.tile([128, CV], F32, tag="mm")
                    for k in range(KT):
                        nc.tensor.matmul(ps[:], xr[k][:, sub * 128:(sub + 1) * 128],
                                         wv[k][:], start=(k == 0), stop=(k == KT - 1))
                    vt = V[t128]
                    v3 = vt[:].rearrange("p (h e) -> p h e", e=DH + 1)
                    nc.vector.tensor_copy(v3[:, :, 0:DH],
                                          ps[:].rearrange("p (h e) -> p h e", e=DH))
                    nc.vector.tensor_copy(v3[:, :, DH], ones128[:])

        # ---- phase B: attention -------------------------------------------
        for gi in range(NTS):
            for hp in range(2):
                njt = 4 * (gi + 1)
                av_lo = ps_av.tile([65, TS], F32, tag="av")
                av_hi = ps_av.tile([65, TS], F32, tag="av")
                for jt in range(njt):
                    ss = ps_s.tile([128, 2 * TS], F32, tag="ss")
                    for half in range(2):
                        p0 = half * 64
                        nc.tensor.matmul(
                            ss[:, half * TS:(half + 1) * TS],
                            qkT[2 + hp][p0:p0 + 64, jt * 128:(jt + 1) * 128],
                            qkT[hp][p0:p0 + 64, gi * TS:(gi + 1) * TS],
                            start=True, stop=True)
                    u = upool.tile([128, 2 * TS], F32R, tag="u")
                    nc.scalar.activation(u[:], ss[:], EXP)
                    d = jt * 128 - gi * TS
                    if d >= 0:  # diagonal tile: apply causal mask
                        mwin = mask_sb[:, 384 - d:384 - d + TS]
                        nc.vector.tensor_tensor(u[:, 0:TS], u[:, 0:TS], mwin, op=MUL)
                        nc.vector.tensor_tensor(u[:, TS:2 * TS], u[:, TS:2 * TS],
                                                mwin, op=MUL)
                    first, last = jt == 0, jt == njt - 1
                    h0, h1 = 2 * hp, 2 * hp + 1
                    nc.tensor.matmul(av_lo[:], V[jt][:, h0 * 65:(h0 + 1) * 65],
                                     u[:, 0:TS], start=first, stop=last)
                    nc.tensor.matmul(av_hi[:], V[jt][:, h1 * 65:(h1 + 1) * 65],
                                     u[:, TS:2 * TS], start=first, stop=last)

                for sub, av in ((0, av_lo), (1, av_hi)):
                    atu = atup.tile([65, TS], F32R, tag="atu")
                    nc.scalar.copy(atu[:], av[:])
                    r = rows.tile([1, TS], F32, tag="r")
                    nc.vector.reciprocal(r[:], atu[64:65, :])
                    rr = rows.tile([1, TS], F32R, tag="rr")
                    nc.vector.tensor_copy(rr[:], r[:])
                    ps_b = ps_mm.tile([64, TS], F32, tag="mm")
                    nc.tensor.matmul(ps_b[:], ones_r[:], rr[:], start=True, stop=True)
                    rbc = rbcp.tile([64, TS], F32, tag="rbc")
                    nc.vector.tensor_copy(rbc[:], ps_b[:])
                    nc.vector.tensor_tensor(
                        aT[hp][sub * 64:(sub + 1) * 64, gi * TS:(gi + 1) * TS],
                        atu[0:64, :], rbc[:], op=MUL)

        # ---- phase C: partial c_proj --------------------------------------
        for tt in range(NT128):
            for nt in range(2):
                ps = ps_mm.tile([128, TS], F32, tag="mm")
                for c in range(2):
                    nc.tensor.matmul(ps[:], aT[c][:, tt * 128:(tt + 1) * 128],
                                     wp[c][:, nt * TS:(nt + 1) * TS],
                                     start=(c == 0), stop=(c == 1))
                o = outp.tile([128, TS], F32, tag="o")
                nc.scalar.copy(o[:], ps[:])
                nc.sync.dma_start(
                    out=out_dram[tt * 128:(tt + 1) * 128, nt * TS:(nt + 1) * TS],
                    in_=o[:])

    nc.finalize()
    return nc


_NC = None


def _get_nc():
    global _NC
    if _NC is None:
        _NC = build()
    return _NC


def _make_in_maps(x, W_attn, b_attn, W_proj):
    jj = np.arange(128, dtype=np.int64)[:, None]
    cc = np.arange(896, dtype=np.int64)[None, :]
    mask = (jj <= cc - 384).astype(np.float32)

    in_maps = []
    for b in range(B):
        xT = np.ascontiguousarray(x[b].T).reshape(KT, 128, T)
        for g in range(4):
            q_cols = W_attn[:, g * CV:(g + 1) * CV]
            k_cols = W_attn[:, D + g * CV:D + (g + 1) * CV]
            wqk = np.ascontiguousarray(
                np.concatenate([q_cols, k_cols], axis=1)).reshape(KT, 128, CQK)
            wv = np.ascontiguousarray(
                W_attn[:, 2 * D + g * CV:2 * D + (g + 1) * CV]).reshape(KT, 128, CV)
            wp = np.ascontiguousarray(
                W_proj[g * CV:(g + 1) * CV, :]).reshape(2, 128, D)
            bq = b_attn[g * CV:(g + 1) * CV] / 8.0
            bk = b_attn[D + g * CV:D + (g + 1) * CV]
            bqk = np.ascontiguousarray(
                np.concatenate([bq, bk]).reshape(4, 128).T).astype(np.float32)
            in_maps.append({
                "xT": xT, "wqk": wqk, "wv": wv, "wp": wp,
                "bqk": bqk, "mask": mask,
            })
    return in_maps


def run(inputs, trace=False):
    x = np.asarray(inputs["x"], dtype=np.float32)
    W_attn = np.asarray(inputs["W_attn"], dtype=np.float32)
    b_attn = np.asarray(inputs["b_attn"], dtype=np.float32)
    W_proj = np.asarray(inputs["W_proj"], dtype=np.float32)
    b_proj = np.asarray(inputs["b_proj"], dtype=np.float32)

    nc = _get_nc()
    in_maps = _make_in_maps(x, W_attn, b_attn, W_proj)
    res = run_bass_kernel_spmd(nc, in_maps, list(range(8)), trace=trace)

    out = np.zeros((B, T, D), dtype=np.float32)
    for b in range(B):
        for g in range(4):
            out[b] += res.results[b * 4 + g]["out"]
    # v-bias contributes a constant shift through the value path; b_proj too.
    const = b_attn[2 * D:3 * D] @ W_proj + b_proj
    out += const[None, None, :].astype(np.float32)
    return out, res


def kernel(**inputs):
    out, _ = run(inputs, trace=False)
    return out
